# revision 26
# baseline (speedup 1.0000x reference)
"""Trainium2 Bass kernel for nn_Decoder_lfa (RandLA-Net style decoder attention layer).

Sharding: 8 cores = (batch b in {0,1}) x (query shard qc in {0..3}, 4096 queries each).
Device layout: channel-major activations [c, rows]; d_att=138 split as A(128)+B(10),
B-channels packed 4x across PE column groups. Gathers via GPSIMD indirect_copy.
Cross-query f_tilde gather: AllGather collective over 4-core replica groups.
"""

import sys

sys.path.insert(0, "/opt/trn_rl_repo")

import numpy as np
from contextlib import ExitStack

import concourse.bass as bass
import concourse.tile as tile
from concourse import bacc, mybir
from concourse.bass_utils import run_bass_kernel_spmd

FP32 = mybir.dt.float32
BF16 = mybir.dt.bfloat16
U16 = mybir.dt.uint16
AF = mybir.ActivationFunctionType
ALU = mybir.AluOpType
EPS = 1e-5

# Full-problem constants
B, QFULL, K, ND, NS = 2, 16384, 16, 4096, 16384
NCORES = 8
NGROUP = 4            # cores per batch (replica group size)
NQ = QFULL // NGROUP  # queries per core = 4096
QT = 256              # queries per tile
R1 = QT * 2 * K       # rows per tile, attention 1 (8192)
R2 = QT * K           # rows per tile, attention 2 (4096)
RSET = QT * K         # rows per gather set per tile (4096)
GCH = RSET // 8       # rows per gpsimd group chunk (512)


def build_nc(nq=NQ, qfull=QFULL, nd=ND, ns=NS, ngroup=NGROUP, ncores=NCORES):
    """Build the single-core SPMD Bass program (same program on all cores)."""
    nt = nq // QT
    nc = bacc.Bacc("TRN2", num_devices=ncores, target_bir_lowering=False, debug=False)

    def din(name, shape, dt):
        return nc.dram_tensor(name, shape, dt, kind="ExternalInput")

    a1_ext = din("a1", [128, nq * 2 * K], BF16)
    tab_a_ext = din("tab_a", [128, ns], BF16)   # sc xyz @16g+{0:3}, lb xyz @16g+{3:6}
    tab_b_ext = din("tab_b", [128, ns], BF16)   # cl xyz @16g+{0:3}
    qxs_ext = din("qxs", [128, nq * K // 8], BF16)
    iw_lb_ext = din("iw_lb", [128, nt * GCH // 16], U16)
    iw_sc_ext = din("iw_sc", [128, nt * GCH // 16], U16)
    iw_cl_ext = din("iw_cl", [128, nt * GCH // 16], U16)
    iw_f_ext = din("iw_f", [128, nq * K // 16], U16)

    wt_aa = din("wt_aa", [128, 128], BF16)
    wt_ba = din("wt_ba", [128, 128], BF16)
    wt_ab = din("wt_ab", [128, 32], BF16)
    wt_bb = din("wt_bb", [128, 32], BF16)
    wt2_aa = din("wt2_aa", [128, 128], BF16)
    wt2_ba = din("wt2_ba", [128, 128], BF16)
    wt2_ab = din("wt2_ab", [128, 32], BF16)
    wt2_bb = din("wt2_bb", [128, 32], BF16)
    wm1_a = din("wm1_a", [128, 128], BF16)
    wm1_b = din("wm1_b", [10, 128], BF16)
    wm2_a = din("wm2_a", [128, 128], BF16)
    wm2_b = din("wm2_b", [10, 128], BF16)
    wlb = din("wlb", [7, 32], BF16)
    wsclo = din("wsclo", [7, 32], BF16)
    wschi = din("wschi", [7, 64], BF16)
    wcl = din("wcl", [7, 32], BF16)
    sel_sc = din("sel_sc", [128, 8], BF16)
    sel_lb = din("sel_lb", [128, 8], BF16)
    selb1 = din("selb1", [128, 10], BF16)
    selb2 = din("selb2", [128, 10], BF16)
    selbb = din("selbb", [128, 40], BF16)
    zrow = din("zrow", [1, 128], BF16)

    bias_ba = din("bias_ba", [128, 1], FP32)
    bias_schi = din("bias_schi", [64, 1], FP32)
    bias_cl = din("bias_cl", [128, 1], FP32)
    b1v = din("b1v", [128, 1], FP32)
    b2v = din("b2v", [128, 1], FP32)

    out_ext = nc.dram_tensor("out", [128, nq], FP32, kind="ExternalOutput")

    with tile.TileContext(nc) as tc, ExitStack() as ctx:
        const_pool = ctx.enter_context(tc.tile_pool(name="const", bufs=1))
        tab_pool = ctx.enter_context(tc.tile_pool(name="tab", bufs=1))
        a1_pool = ctx.enter_context(tc.tile_pool(name="a1p", bufs=2))
        b1_pool = ctx.enter_context(tc.tile_pool(name="b1p", bufs=2))
        e10_pool = ctx.enter_context(tc.tile_pool(name="e10p", bufs=2))
        sprd_pool = ctx.enter_context(tc.tile_pool(name="sprd", bufs=4))
        eh_pool = ctx.enter_context(tc.tile_pool(name="ehp", bufs=8))
        fold_pool = ctx.enter_context(tc.tile_pool(name="foldp", bufs=3))
        z_pool = ctx.enter_context(tc.tile_pool(name="zp", bufs=2))
        f_pool = ctx.enter_context(tc.tile_pool(name="fp", bufs=1))
        out_pool = ctx.enter_context(tc.tile_pool(name="outp", bufs=2))
        dram_pool = ctx.enter_context(tc.tile_pool(name="dram", bufs=1, space="DRAM"))

        ps_a = ctx.enter_context(tc.tile_pool(name="ps_a", bufs=2, space="PSUM"))
        ps_b = ctx.enter_context(tc.tile_pool(name="ps_b", bufs=1, space="PSUM"))
        ps_c = ctx.enter_context(tc.tile_pool(name="ps_c", bufs=2, space="PSUM"))

        sync = nc.sync

        def const_tile(ext):
            t = const_pool.tile(list(ext.shape), ext.dtype, name=ext.name + "_sb")
            sync.dma_start(t[:], ext[:])
            return t

        WT_AA = const_tile(wt_aa); WT_BA = const_tile(wt_ba)
        WT_AB = const_tile(wt_ab); WT_BB = const_tile(wt_bb)
        WT2_AA = const_tile(wt2_aa); WT2_BA = const_tile(wt2_ba)
        WT2_AB = const_tile(wt2_ab); WT2_BB = const_tile(wt2_bb)
        WM1_A = const_tile(wm1_a); WM1_B = const_tile(wm1_b)
        WM2_A = const_tile(wm2_a); WM2_B = const_tile(wm2_b)
        WLB = const_tile(wlb); WSCLO = const_tile(wsclo)
        WSCHI = const_tile(wschi); WCL = const_tile(wcl)
        SEL_SC = const_tile(sel_sc); SEL_LB = const_tile(sel_lb)
        SELB1 = const_tile(selb1); SELB2 = const_tile(selb2)
        SELBB = const_tile(selbb); ZROW = const_tile(zrow)
        BIAS_BA = const_tile(bias_ba); BIAS_SCHI = const_tile(bias_schi)
        BIAS_CL = const_tile(bias_cl); B1V = const_tile(b1v); B2V = const_tile(b2v)
        QXS = const_tile(qxs_ext)
        IWLB = const_tile(iw_lb_ext); IWSC = const_tile(iw_sc_ext)
        IWCL = const_tile(iw_cl_ext); IWF = const_tile(iw_f_ext)

        TABA = tab_pool.tile([128, ns], BF16, name="taba", tag="tab")
        sync.dma_start(TABA[:], tab_a_ext[:])

        F = f_pool.tile([128, nq], BF16, name="ftilde", tag="ff")

        # ---- helper: rel-pos encoding -> E7 [7, RSET] bf16 for one set ----
        # E7 channels: [dis, rel_x, nbr_x, rel_y, nbr_y, rel_z, nbr_z]
        # (the reference's `tile` q-channels are folded into W: q = rel + nbr)
        def relpos(t, tab_tile, iw_tile, cb, sel_tile, nsrc):
            rn = sprd_pool.tile([128, 2 * GCH], BF16, name="rn", tag="sp")
            nc.gpsimd.indirect_copy(
                rn[:, GCH:2 * GCH], tab_tile[:, :nsrc],
                iw_tile[:, t * (GCH // 16):(t + 1) * (GCH // 16)],
                i_know_ap_gather_is_preferred=True)
            qx = QXS[:, t * GCH:(t + 1) * GCH]
            nc.vector.tensor_sub(rn[:, 0:GCH], qx, rn[:, GCH:2 * GCH])
            rel2 = sprd_pool.tile([128, GCH], BF16, name="rel2", tag="sp2")
            nc.vector.tensor_mul(rel2[:], rn[:, 0:GCH], rn[:, 0:GCH])
            dps = ps_c.tile([8, GCH], FP32, name="dps", tag="psc")
            nc.tensor.matmul(dps[:], sel_tile[:, :], rel2[:, :], start=True, stop=True)
            dis = sprd_pool.tile([8, GCH], BF16, name="dis", tag="spd")
            nc.scalar.activation(dis[:], dps[:], AF.Sqrt)
            e7 = e10_pool.tile([7, RSET], BF16, name="e7", tag="e10")
            sync.dma_start(e7[0:1, :], dis[:, :])
            for g in range(8):
                nc.scalar.dma_start(
                    e7[1:7, GCH * g:GCH * (g + 1)],
                    rn[16 * g + cb:16 * g + cb + 3, :].rearrange(
                        "p (t f) -> p t f", t=2))
            return e7

        # ---- helper: fold [128, n*16] (viewed [., n, 16]) -> dst_ap [128, n] ----
        def fold16(src, n, dst_ap):
            v = src.rearrange("p (q k) -> p q k", k=16)
            f2 = fold_pool.tile([128, n * 8], BF16, name="f2", tag="fold")
            f2v = f2.rearrange("p (q k) -> p q k", k=8)
            nc.vector.tensor_add(f2v[:, :, :], v[:, :, 0:8], v[:, :, 8:16])
            f3 = fold_pool.tile([128, n * 4], BF16, name="f3", tag="fold")
            f3v = f3.rearrange("p (q k) -> p q k", k=4)
            nc.vector.tensor_add(f3v[:, :, :], f2v[:, :, 0:4], f2v[:, :, 4:8])
            f4 = fold_pool.tile([128, n * 2], BF16, name="f4", tag="fold")
            f4v = f4.rearrange("p (q k) -> p q k", k=2)
            nc.vector.tensor_add(f4v[:, :, :], f3v[:, :, 0:2], f3v[:, :, 2:4])
            nc.vector.tensor_add(dst_ap, f4v[:, :, 0], f4v[:, :, 1])

        # ================= PHASE A =================
        for t in range(nt):
            a1 = a1_pool.tile([128, R1], BF16, name="a1t", tag="a1")
            sync.dma_start(a1[:, :], a1_ext[:, t * R1:(t + 1) * R1])

            e_lb = relpos(t, TABA, IWLB, 3, SEL_LB, nd)
            e_sc = relpos(t, TABA, IWSC, 0, SEL_SC, ns)

            b1s = b1_pool.tile([128, R1 // 4], BF16, name="b1s", tag="b1")
            for m in range(4):
                pc = ps_c.tile([128, 512], FP32, name="convp", tag="psc")
                plan = ((e_lb, WLB), (e_lb, WLB), (e_sc, WSCLO), (e_sc, WSCLO))
                for jj, (esrc, wconv) in enumerate(plan):
                    w = (jj % 2) * 4 + m
                    nc.tensor.matmul(pc[32 * jj:32 * (jj + 1), :], wconv[:, :],
                                     esrc[:, 512 * w:512 * (w + 1)],
                                     start=True, stop=True,
                                     tile_position=(0, 32 * jj))
                nc.scalar.activation(b1s[:, 512 * m:512 * (m + 1)], pc[:, :], AF.Relu,
                                     bias=BIAS_BA[:, :])
            for w in range(8):
                ph = ps_c.tile([64, 512], FP32, name="schip", tag="psc")
                nc.tensor.matmul(ph[:, :], WSCHI[:, :], e_sc[:, 512 * w:512 * (w + 1)],
                                 start=True, stop=True, tile_position=(0, 0))
                nc.scalar.activation(a1[0:64, RSET + 512 * w:RSET + 512 * (w + 1)],
                                     ph[:, :], AF.Relu, bias=BIAS_SCHI[:, :])

            # ---- attention 1 ----
            za = z_pool.tile([128, QT], FP32, name="za", tag="za", bufs=4)
            ha = z_pool.tile([128, QT], FP32, name="ha", tag="za", bufs=4)
            zbs = z_pool.tile([128, 128], BF16, name="zbs", tag="zb", bufs=4)
            hbs = z_pool.tile([128, 128], BF16, name="hbs", tag="zb", bufs=4)
            for mp in range(2):
                ehs = {}
                for j in (0, 2, 1, 3):
                    pa = ps_a.tile([128, 1024], FP32, name="pa", tag="psa")
                    for h in range(2):
                        col = 2048 * j + 1024 * mp + 512 * h
                        bcol = 1024 * mp + 512 * h
                        nc.tensor.matmul(pa[:, 512 * h:512 * (h + 1)], WT_AA[:, :],
                                         a1[:, col:col + 512], start=True, stop=False,
                                         tile_position=(0, 0))
                        nc.tensor.matmul(pa[:, 512 * h:512 * (h + 1)],
                                         WT_BA[32 * j:32 * j + 10, :],
                                         b1s[32 * j:32 * j + 10, bcol:bcol + 512],
                                         start=False, stop=True,
                                         tile_position=(32 * j, 0))
                    ee = eh_pool.tile([128, 1024], BF16, name="ee", tag="eh")
                    nc.scalar.activation(ee[:], pa[:], AF.Exp)
                    hh = eh_pool.tile([128, 1024], BF16, name="hh", tag="eh")
                    nc.vector.tensor_mul(
                        hh[:], ee[:],
                        a1[:, 2048 * j + 1024 * mp:2048 * j + 1024 * (mp + 1)])
                    ehs[j] = (ee, hh)
                    if j in (2, 3):
                        ja = j - 2
                        pi = ja
                        c0 = 128 * pi + 64 * mp
                        for src_idx, dst in ((0, za), (1, ha)):
                            fe = fold_pool.tile([128, 1024], BF16, name="fe", tag="fold1")
                            nc.vector.tensor_add(fe[:], ehs[ja][src_idx][:],
                                                 ehs[j][src_idx][:])
                            fold16(fe, 64, dst[:, c0:c0 + 64])
                # B part
                pb = ps_b.tile([128, 1024], FP32, name="pb", tag="psb")
                for j in range(4):
                    for h in range(2):
                        col = 2048 * j + 1024 * mp + 512 * h
                        bcol = 1024 * mp + 512 * h
                        nc.tensor.matmul(pb[32 * j:32 * (j + 1), 512 * h:512 * (h + 1)],
                                         WT_AB[:, :], a1[:, col:col + 512],
                                         start=True, stop=False,
                                         tile_position=(0, 32 * j))
                        nc.tensor.matmul(pb[32 * j:32 * (j + 1), 512 * h:512 * (h + 1)],
                                         WT_BB[32 * j:32 * j + 10, :],
                                         b1s[32 * j:32 * j + 10, bcol:bcol + 512],
                                         start=False, stop=True,
                                         tile_position=(32 * j, 32 * j))
                ebs = eh_pool.tile([128, 1024], BF16, name="ebs", tag="eh")
                nc.scalar.activation(ebs[:], pb[:], AF.Exp)
                hbsm = eh_pool.tile([128, 1024], BF16, name="hbsm", tag="eh")
                nc.vector.tensor_mul(hbsm[:], ebs[:], b1s[:, 1024 * mp:1024 * (mp + 1)])
                fold16(ebs, 64, zbs[:, 64 * mp:64 * mp + 64])
                fold16(hbsm, 64, hbs[:, 64 * mp:64 * mp + 64])
            zbp = ps_c.tile([10, 256], FP32, name="zbp", tag="psc")
            hbp = ps_c.tile([10, 256], FP32, name="hbp", tag="psc")
            nc.tensor.matmul(zbp[:, 0:128], SELB1[:, :], zbs[:, :], start=True, stop=True,
                             tile_position=(0, 0))
            nc.tensor.matmul(zbp[:, 128:256], SELB2[:, :], zbs[:, :], start=True, stop=True,
                             tile_position=(0, 0))
            nc.tensor.matmul(hbp[:, 0:128], SELB1[:, :], hbs[:, :], start=True, stop=True,
                             tile_position=(0, 0))
            nc.tensor.matmul(hbp[:, 128:256], SELB2[:, :], hbs[:, :], start=True, stop=True,
                             tile_position=(0, 0))
            rza = z_pool.tile([128, QT], FP32, name="rza", tag="rz", bufs=2)
            scr = z_pool.tile([128, QT], FP32, name="scr", tag="rz", bufs=2)
            nc.vector.reciprocal_approx_accurate(rza[:], za[:], scr[:])
            agA = z_pool.tile([128, QT], BF16, name="agA", tag="ag", bufs=2)
            nc.vector.tensor_mul(agA[:], ha[:], rza[:])
            rzb = z_pool.tile([10, 256], FP32, name="rzb", tag="rzb", bufs=4)
            scrb = z_pool.tile([10, 256], FP32, name="scrb", tag="rzb", bufs=4)
            nc.vector.reciprocal_approx_accurate(rzb[:], zbp[:], scrb[:])
            agB = z_pool.tile([10, 256], BF16, name="agB", tag="rzb", bufs=4)
            nc.vector.tensor_mul(agB[:], hbp[:], rzb[:])
            pm = ps_c.tile([128, 256], FP32, name="pm", tag="psc")
            nc.tensor.matmul(pm[:, :], WM1_A[:, :], agA[:, :], start=True, stop=False,
                             tile_position=(0, 0))
            nc.tensor.matmul(pm[:, :], WM1_B[:, :], agB[:, :], start=False, stop=True,
                             tile_position=(0, 0))
            nc.scalar.activation(F[:, t * QT:(t + 1) * QT], pm[:, :], AF.Relu,
                                 bias=B1V[:, :])

        # ================= collective: allgather f_tilde =================
        cc_in = dram_pool.tile([128, nq], BF16, name="cc_in", tag="ccin")
        sync.dma_start(cc_in[:, :], F[:, :])
        cc_out = dram_pool.tile([ngroup, 128, nq], BF16, name="cc_out", tag="ccout")
        groups = [[g * ngroup + i for i in range(ngroup)]
                  for g in range(ncores // ngroup)]
        nc.gpsimd.collective_compute(
            "AllGather", ALU.bypass, replica_groups=groups,
            ins=[cc_in.opt()], outs=[cc_out.opt()])
        FFULL = f_pool.tile([128, qfull], BF16, name="ffull", tag="ff")
        for g in range(ngroup):
            sync.dma_start(FFULL[:, g * nq:(g + 1) * nq], cc_out[g])

        TABB = tab_pool.tile([128, ns], BF16, name="tabb", tag="tab")
        sync.dma_start(TABB[:], tab_b_ext[:])

        # ================= PHASE B =================
        for t in range(nt):
            a2 = a1_pool.tile([128, R2], BF16, name="a2t", tag="a1")
            for cch in range(R2 // 512):
                nc.gpsimd.indirect_copy(
                    a2[:, 512 * cch:512 * (cch + 1)], FFULL[:, :],
                    IWF[:, t * (R2 // 16) + 32 * cch:t * (R2 // 16) + 32 * (cch + 1)],
                    i_know_ap_gather_is_preferred=True)
            e_cl = relpos(t, TABB, IWCL, 0, SEL_SC, qfull)
            b2s = b1_pool.tile([128, R2 // 4], BF16, name="b2s", tag="b1")
            for m in range(2):
                pc = ps_c.tile([128, 512], FP32, name="convp2", tag="psc")
                for jj in range(4):
                    w = jj * 2 + m
                    nc.tensor.matmul(pc[32 * jj:32 * (jj + 1), :], WCL[:, :],
                                     e_cl[:, 512 * w:512 * (w + 1)],
                                     start=True, stop=True,
                                     tile_position=(0, 32 * jj))
                nc.scalar.activation(b2s[:, 512 * m:512 * (m + 1)], pc[:, :], AF.Relu,
                                     bias=BIAS_CL[:, :])

            za = z_pool.tile([128, QT], FP32, name="za2", tag="za", bufs=4)
            ha = z_pool.tile([128, QT], FP32, name="ha2", tag="za", bufs=4)
            zbs = z_pool.tile([128, 64], BF16, name="zbs2", tag="zb", bufs=4)
            hbs = z_pool.tile([128, 64], BF16, name="hbs2", tag="zb", bufs=4)
            pb = ps_b.tile([128, 1024], FP32, name="pb2", tag="psb")
            for j in range(4):
                pa = ps_a.tile([128, 1024], FP32, name="pa2", tag="psa")
                for h in range(2):
                    col = 1024 * j + 512 * h
                    nc.tensor.matmul(pa[:, 512 * h:512 * (h + 1)], WT2_AA[:, :],
                                     a2[:, col:col + 512], start=True, stop=False,
                                     tile_position=(0, 0))
                    nc.tensor.matmul(pa[:, 512 * h:512 * (h + 1)],
                                     WT2_BA[32 * j:32 * j + 10, :],
                                     b2s[32 * j:32 * j + 10, 512 * h:512 * (h + 1)],
                                     start=False, stop=True, tile_position=(32 * j, 0))
                    nc.tensor.matmul(pb[32 * j:32 * (j + 1), 512 * h:512 * (h + 1)],
                                     WT2_AB[:, :], a2[:, col:col + 512],
                                     start=True, stop=False, tile_position=(0, 32 * j))
                    nc.tensor.matmul(pb[32 * j:32 * (j + 1), 512 * h:512 * (h + 1)],
                                     WT2_BB[32 * j:32 * j + 10, :],
                                     b2s[32 * j:32 * j + 10, 512 * h:512 * (h + 1)],
                                     start=False, stop=True,
                                     tile_position=(32 * j, 32 * j))
                ee = eh_pool.tile([128, 1024], BF16, name="ee2", tag="eh")
                nc.scalar.activation(ee[:], pa[:], AF.Exp)
                hh = eh_pool.tile([128, 1024], BF16, name="hh2", tag="eh")
                nc.vector.tensor_mul(hh[:], ee[:], a2[:, 1024 * j:1024 * (j + 1)])
                fold16(ee, 64, za[:, 64 * j:64 * j + 64])
                fold16(hh, 64, ha[:, 64 * j:64 * j + 64])
            ebs = eh_pool.tile([128, 1024], BF16, name="ebs2", tag="eh")
            nc.scalar.activation(ebs[:], pb[:], AF.Exp)
            hbsm = eh_pool.tile([128, 1024], BF16, name="hbsm2", tag="eh")
            nc.vector.tensor_mul(hbsm[:], ebs[:], b2s[:, :])
            fold16(ebs, 64, zbs[:, 0:64])
            fold16(hbsm, 64, hbs[:, 0:64])
            zbp = ps_c.tile([10, 256], FP32, name="zbp2", tag="psc")
            hbp = ps_c.tile([10, 256], FP32, name="hbp2", tag="psc")
            for j in range(4):
                nc.tensor.matmul(zbp[:, 64 * j:64 * (j + 1)],
                                 SELBB[:, 10 * j:10 * (j + 1)], zbs[:, :],
                                 start=True, stop=True, tile_position=(0, 0))
                nc.tensor.matmul(hbp[:, 64 * j:64 * (j + 1)],
                                 SELBB[:, 10 * j:10 * (j + 1)], hbs[:, :],
                                 start=True, stop=True, tile_position=(0, 0))
            rza = z_pool.tile([128, QT], FP32, name="rza2", tag="rz", bufs=2)
            scr = z_pool.tile([128, QT], FP32, name="scr2", tag="rz", bufs=2)
            nc.vector.reciprocal_approx_accurate(rza[:], za[:], scr[:])
            agA = z_pool.tile([128, QT], BF16, name="agA2", tag="ag", bufs=2)
            nc.vector.tensor_mul(agA[:], ha[:], rza[:])
            rzb = z_pool.tile([10, 256], FP32, name="rzb2", tag="rzb", bufs=4)
            scrb = z_pool.tile([10, 256], FP32, name="scrb2", tag="rzb", bufs=4)
            nc.vector.reciprocal_approx_accurate(rzb[:], zbp[:], scrb[:])
            agB = z_pool.tile([10, 256], BF16, name="agB2", tag="rzb", bufs=4)
            nc.vector.tensor_mul(agB[:], hbp[:], rzb[:])
            pm = ps_c.tile([128, 256], FP32, name="pm2", tag="psc")
            nc.tensor.matmul(pm[:, :], WM2_A[:, :], agA[:, :], start=True, stop=False,
                             tile_position=(0, 0))
            nc.tensor.matmul(pm[:, :], WM2_B[:, :], agB[:, :], start=False, stop=True,
                             tile_position=(0, 0))
            ot = out_pool.tile([128, QT], FP32, name="ot", tag="ot")
            nc.scalar.activation(ot[:], pm[:, :], AF.Relu, bias=B2V[:, :])
            sync.dma_start(out_ext[:, t * QT:(t + 1) * QT], ot[:])

    nc.compile()
    return nc


# =====================================================================
# Host-side data preparation
# =====================================================================

def _wrap_idx(idx_rows):
    n = idx_rows.shape[0]
    return idx_rows.reshape(n // 16, 16).T


def prep_core_inputs(inputs, b, qc, nq=NQ, nd=ND, ns=NS, qfull=QFULL):
    import ml_dtypes
    bf16 = ml_dtypes.bfloat16
    f32 = np.float32
    q0 = qc * nq
    nt = nq // QT

    feat_lb = np.asarray(inputs["features_lb"][b, q0:q0 + nq], f32)
    feat_sc = np.asarray(inputs["features_sc"][b, q0:q0 + nq], f32)
    nm_lb = np.asarray(inputs["NM_lb"][b, q0:q0 + nq]).astype(np.int64)
    nm_sc = np.asarray(inputs["NM_sc"][b, q0:q0 + nq]).astype(np.int64)
    nm_cl = np.asarray(inputs["NM_cl"][b, q0:q0 + nq]).astype(np.int64)
    c_lb = np.asarray(inputs["coords_lb"][b], f32)
    c_sc = np.asarray(inputs["coords_sc"][b], f32)
    c_q = np.asarray(inputs["coords_queries"][b], f32)
    cq_loc = c_q[q0:q0 + nq]

    d = {}
    a1 = np.zeros((128, nq * 2 * K), dtype=bf16)
    a1v = a1.reshape(128, nt, 2, QT * K)
    flb = feat_lb.reshape(nt, QT * K, 128).transpose(0, 2, 1)
    a1v[:, :, 0, :] = flb.transpose(1, 0, 2).astype(bf16)
    fsc = feat_sc.reshape(nt, QT * K, 64).transpose(0, 2, 1)
    a1v[64:128, :, 1, :] = fsc.transpose(1, 0, 2).astype(bf16)
    d["a1"] = a1

    tab_a = np.zeros((128, ns), dtype=bf16)
    tab_b = np.zeros((128, ns), dtype=bf16)
    for g in range(8):
        for c in range(3):
            tab_a[16 * g + c, :ns] = c_sc[:, c].astype(bf16)
            tab_a[16 * g + 3 + c, :nd] = c_lb[:, c].astype(bf16)
            tab_b[16 * g + c, :qfull] = c_q[:, c].astype(bf16)
    d["tab_a"] = tab_a
    d["tab_b"] = tab_b

    qxs = np.zeros((128, nq * K // 8), dtype=bf16)
    rows = np.arange(RSET)
    for g in range(8):
        sel = rows[512 * g:512 * (g + 1)]
        for t in range(nt):
            qv = cq_loc[t * QT + sel // K]
            for c in range(3):
                qxs[16 * g + c, t * 512:(t + 1) * 512] = qv[:, c].astype(bf16)
                qxs[16 * g + 3 + c, t * 512:(t + 1) * 512] = qv[:, c].astype(bf16)
    d["qxs"] = qxs

    def wrap_set(nm):
        out = np.zeros((128, nt * (GCH // 16)), dtype=np.uint16)
        flat = nm.reshape(nt, RSET)
        for t in range(nt):
            for g in range(8):
                chunk = flat[t, 512 * g:512 * (g + 1)].astype(np.uint16)
                out[16 * g:16 * (g + 1), t * 32:(t + 1) * 32] = _wrap_idx(chunk)
        return out

    d["iw_lb"] = wrap_set(nm_lb)
    d["iw_sc"] = wrap_set(nm_sc)
    d["iw_cl"] = wrap_set(nm_cl)
    iwf = np.zeros((128, nq * K // 16), dtype=np.uint16)
    flat = nm_cl.reshape(nt, R2)
    for t in range(nt):
        w = _wrap_idx(flat[t].astype(np.uint16))
        for g in range(8):
            iwf[16 * g:16 * (g + 1), t * (R2 // 16):(t + 1) * (R2 // 16)] = w
    d["iw_f"] = iwf

    perm = np.concatenate([np.arange(10, 138), np.arange(10)])
    W1 = np.asarray(inputs["W_fc1"], f32)[perm][:, perm]
    W2 = np.asarray(inputs["W_fc2"], f32)[perm][:, perm]
    Wm1 = np.asarray(inputs["W_mlp1"], f32)[:, perm]
    Wm2 = np.asarray(inputs["W_mlp2"], f32)[:, perm]

    def score_w(W, pfx):
        WT = W.T.astype(f32)
        d[pfx + "_aa"] = WT[0:128, 0:128].astype(bf16)
        ba = np.zeros((128, 128), dtype=bf16)
        bb = np.zeros((128, 32), dtype=bf16)
        ab = np.zeros((128, 32), dtype=bf16)
        ab[:, 0:10] = WT[0:128, 128:138].astype(bf16)
        for j in range(4):
            ba[32 * j:32 * j + 10, :] = WT[128:138, 0:128].astype(bf16)
            bb[32 * j:32 * j + 10, 0:10] = WT[128:138, 128:138].astype(bf16)
        d[pfx + "_ba"] = ba
        d[pfx + "_ab"] = ab
        d[pfx + "_bb"] = bb

    score_w(W1, "wt")
    score_w(W2, "wt2")

    def to_e7(W):
        # [dis, rel, q, nbr] (10 cols) -> [dis, rx, nx, ry, ny, rz, nz] (7 cols)
        # using q = rel + nbr
        W7 = np.zeros((W.shape[0], 7), np.float32)
        W7[:, 0] = W[:, 0]
        for i in range(3):
            W7[:, 1 + 2 * i] = W[:, 1 + i] + W[:, 4 + i]
            W7[:, 2 + 2 * i] = W[:, 4 + i] + W[:, 7 + i]
        return W7

    def bnfold(W, g, bb_, m, v):
        gp = g / np.sqrt(v + EPS)
        bp = bb_ - m * gp
        return (to_e7(W) * gp[:, None]).astype(f32).T, bp

    wlbT, blb = bnfold(np.asarray(inputs["W_lb"], f32),
                       *[np.asarray(inputs[k], f32) for k in ("g_lb", "b_lb", "m_lb", "v_lb")])
    wscT, bsc = bnfold(np.asarray(inputs["W_sc"], f32),
                       *[np.asarray(inputs[k], f32) for k in ("g_sc", "b_sc", "m_sc", "v_sc")])
    wclT, bcl = bnfold(np.asarray(inputs["W_cl"], f32),
                       *[np.asarray(inputs[k], f32) for k in ("g_cl", "b_cl", "m_cl", "v_cl")])
    def pad32(w):
        out = np.zeros((7, 32), dtype=bf16)
        out[:, :w.shape[1]] = w.astype(bf16)
        return out

    d["wlb"] = pad32(wlbT)
    d["wsclo"] = pad32(wscT[:, 0:10])
    d["wschi"] = wscT[:, 10:74].astype(bf16)
    d["wcl"] = pad32(wclT)

    bias_ba = np.zeros((128, 1), f32)
    for j in range(4):
        bias_ba[32 * j:32 * j + 10, 0] = blb if j < 2 else bsc[0:10]
    d["bias_ba"] = bias_ba
    d["bias_schi"] = bsc[10:74].reshape(64, 1).astype(f32)
    bias_cl = np.zeros((128, 1), f32)
    for j in range(4):
        bias_cl[32 * j:32 * j + 10, 0] = bcl
    d["bias_cl"] = bias_cl

    def bnvec(g, bb_, m, v):
        gp = g / np.sqrt(v + EPS)
        return (bb_ - m * gp), gp

    b1p, g1p = bnvec(*[np.asarray(inputs[k], f32) for k in ("g1", "b1", "m1", "v1")])
    b2p, g2p = bnvec(*[np.asarray(inputs[k], f32) for k in ("g2", "b2", "m2", "v2")])
    d["wm1_a"] = (Wm1.T[0:128, :] * g1p[None, :]).astype(bf16)
    d["wm1_b"] = (Wm1.T[128:138, :] * g1p[None, :]).astype(bf16)
    d["wm2_a"] = (Wm2.T[0:128, :] * g2p[None, :]).astype(bf16)
    d["wm2_b"] = (Wm2.T[128:138, :] * g2p[None, :]).astype(bf16)
    d["b1v"] = b1p.reshape(128, 1).astype(f32)
    d["b2v"] = b2p.reshape(128, 1).astype(f32)

    sel_sc = np.zeros((128, 8), dtype=bf16)
    sel_lb = np.zeros((128, 8), dtype=bf16)
    for g in range(8):
        for c in range(3):
            sel_sc[16 * g + c, g] = 1
            sel_lb[16 * g + 3 + c, g] = 1
    d["sel_sc"] = sel_sc
    d["sel_lb"] = sel_lb
    selb1 = np.zeros((128, 10), dtype=bf16)
    selb2 = np.zeros((128, 10), dtype=bf16)
    selbb = np.zeros((128, 40), dtype=bf16)
    for c in range(10):
        selb1[c, c] = 1; selb1[64 + c, c] = 1
        selb2[32 + c, c] = 1; selb2[96 + c, c] = 1
        for j in range(4):
            selbb[32 * j + c, 10 * j + c] = 1
    d["selb1"] = selb1
    d["selb2"] = selb2
    d["selbb"] = selbb
    d["zrow"] = np.zeros((1, 128), dtype=bf16)
    return d


_CACHE = {}


def _run(inputs, trace=False, trace_kwargs=None):
    if "nc" not in _CACHE:
        _CACHE["nc"] = build_nc()
    nc = _CACHE["nc"]
    in_maps = []
    for core in range(NCORES):
        b, qc = core // NGROUP, core % NGROUP
        in_maps.append(prep_core_inputs(inputs, b, qc))
    res = run_bass_kernel_spmd(nc, in_maps, list(range(NCORES)),
                               trace=trace, **(trace_kwargs or {}))
    out = np.zeros((B, 128, QFULL, 1), np.float32)
    for core in range(NCORES):
        b, qc = core // NGROUP, core % NGROUP
        out[b, :, qc * NQ:(qc + 1) * NQ, 0] = res.results[core]["out"]
    return out, res


def kernel(**inputs) -> np.ndarray:
    return _run(inputs, trace=False)[0]
